# revision 12
# baseline (speedup 1.0000x reference)
"""GPTQ 4-bit quantized linear (CaiQuantLinear) on 8 TRN2 NeuronCores.

Computes out = x @ dequant(qweight, scales, qzeros) + bias where
  x: (4, 2048, 4096) fp16, qweight: (512, 4096) int32 (8x 4-bit per word,
  packed along input features), scales: (32, 4096) fp16, qzeros: (32, 512)
  int32 (packed along output features), bias: (4096,) fp16.
  Groups are contiguous blocks of 128 input features (g_idx = arange//128).

Sharding: tensor-parallel column split over output features. Each of the 8
cores gets 512 output columns (its slice of qweight/scales/qzeros/bias) and
the full x (replicated). No collectives; the host concatenates the 8 column
slices.

Per-core kernel:
  1. Unpack nibble planes of qweight in natural layout ([word-row, out]),
     dequantize against partition-broadcast (z+1) and scale tiles.
  2. Fix the partition permutation (plane s, word-row r -> input feature
     8r+s) with two PE transposes (planes -> w^T -> w), leaving fp16
     weights resident in SBUF as [128, 32 k-tiles, 512 out].
  3. Stream x through DMA-transpose into [128 in, seq-chunk] tiles and
     accumulate 32 matmuls per 128-seq output tile into PSUM; add bias on
     the way out.
"""

import sys

if "/opt/trn_rl_repo" not in sys.path:
    sys.path.insert(0, "/opt/trn_rl_repo")

import numpy as np

B, S, IN, OUT = 4, 2048, 4096, 4096
SEQ = B * S                      # 8192
NCORES = 8
OUT_S = OUT // NCORES            # 512 output columns per core
PACK = 8                         # int32 packs 8 nibbles
GSIZE = 128                      # group size == k-tile size

_CACHE = {}


def _build(seq, in_f, out_s, chunk):
    """Build + compile the per-core Bass program. All cores run the same
    NEFF on their own input slices (SPMD, no collectives)."""
    from contextlib import ExitStack  # noqa: F401

    import concourse.bass as bass  # noqa: F401
    import concourse.mybir as mybir
    import concourse.tile as tile
    from concourse import bacc
    from concourse.masks import make_identity

    dt = mybir.dt
    op = mybir.AluOpType
    P = 128
    KT = in_f // P                # k-tiles (== groups)
    QR = in_f // PACK             # qweight rows
    RT = QR // P                  # qweight row-tiles
    OT = out_s // P               # 128-wide output blocks per core
    NCH = seq // chunk            # seq chunks
    ST = chunk // P               # seq tiles per chunk

    nc = bacc.Bacc("TRN2", target_bir_lowering=False, debug=False,
                   num_devices=NCORES)

    x_d = nc.dram_tensor("x", (seq, in_f), dt.float16, kind="ExternalInput")
    qw_d = nc.dram_tensor("qweight", (QR, out_s), dt.int32, kind="ExternalInput")
    sc_d = nc.dram_tensor("scales", (KT, out_s), dt.float16, kind="ExternalInput")
    qz_d = nc.dram_tensor("qzeros", (KT, out_s // PACK), dt.int32,
                          kind="ExternalInput")
    b_d = nc.dram_tensor("bias", (1, out_s), dt.float16, kind="ExternalInput")
    out_d = nc.dram_tensor("out", (seq, out_s), dt.float16, kind="ExternalOutput")

    x = x_d.ap()
    qw = qw_d.ap()
    scales = sc_d.ap()
    qzeros = qz_d.ap()
    bias = b_d.ap()
    out = out_d.ap()

    with tile.TileContext(nc) as tc:
        with (
            tc.tile_pool(name="const", bufs=1) as const_pool,
            tc.tile_pool(name="w", bufs=1) as w_pool,
            tc.tile_pool(name="qst", bufs=3) as q_pool,
            tc.tile_pool(name="plane", bufs=2) as plane_pool,
            tc.tile_pool(name="wt", bufs=2) as wt_pool,
            tc.tile_pool(name="bc", bufs=2) as bc_pool,
            tc.tile_pool(name="xt", bufs=52) as xt_pool,
            tc.tile_pool(name="ot", bufs=4) as out_pool,
            tc.tile_pool(name="ps", bufs=4, space="PSUM") as psum_pool,
            tc.tile_pool(name="pst", bufs=2, space="PSUM") as psumt_pool,
            tc.tile_pool(name="dram", bufs=1, space="DRAM") as dram_pool,
        ):
            # ---- constants ----
            ident = const_pool.tile([P, P], dt.float16)
            make_identity(nc, ident)

            bias16 = const_pool.tile([P, out_s], dt.float16)
            nc.gpsimd.dma_start(bias16, bias.to_broadcast((P, out_s)))
            bias32 = const_pool.tile([P, out_s], dt.float32)
            nc.vector.tensor_copy(bias32, bias16)

            # ---- dequantize weights ----
            # w_all[:, k, :]: k-tile k of fp16 weights, [128 in x out_s]
            w_all = w_pool.tile([P, KT, out_s], dt.float16)
            for rt in range(RT):
                q_nat = q_pool.tile([P, out_s], dt.int32, tag="qnat")
                nc.gpsimd.dma_start(q_nat, qw[rt * P:(rt + 1) * P, :])

                # broadcast (z+1) and scale rows for this row-tile:
                # partition wr needs group 8*rt + wr//16. qzeros is loaded
                # directly in broadcast layout and unpacked on-chip.
                qzb = bc_pool.tile([P, out_s // PACK], dt.int32, tag="qzb")
                nc.gpsimd.dma_start(
                    qzb,
                    qzeros[8 * rt:8 * rt + 8][:, None, :]
                    .to_broadcast((8, 16, out_s // PACK)))
                zb_i = bc_pool.tile([P, out_s], dt.int32, tag="zbi")
                zbv = zb_i.rearrange("p (c s) -> p c s", s=PACK)
                for s in range(PACK):
                    nc.vector.tensor_scalar(
                        out=zbv[:, :, s], in0=qzb, scalar1=4 * s, scalar2=0xF,
                        op0=op.logical_shift_right, op1=op.bitwise_and)
                z1_bc = bc_pool.tile([P, out_s], dt.float16, tag="z1bc")
                nc.vector.tensor_scalar_add(z1_bc, zb_i, 1.0)
                s_bc = bc_pool.tile([P, out_s], dt.float16, tag="sbc")
                nc.gpsimd.dma_start(
                    s_bc,
                    scales[8 * rt:8 * rt + 8][:, None, :]
                    .to_broadcast((8, 16, out_s)))

                # unpack 8 nibble planes + dequant (still [word-row, out])
                planes = plane_pool.tile([P, PACK, out_s], dt.float16, tag="pl")
                for s in range(PACK):
                    plane_i = q_pool.tile([P, out_s], dt.int32, tag="plane_i")
                    nc.vector.tensor_scalar(
                        out=plane_i, in0=q_nat, scalar1=4 * s,
                        scalar2=0xF, op0=op.logical_shift_right,
                        op1=op.bitwise_and)
                    # fused cast + subtract: (plane + 0) - (z+1)
                    nc.vector.scalar_tensor_tensor(
                        out=planes[:, s, :], in0=plane_i, scalar=0.0,
                        in1=z1_bc, op0=op.add, op1=op.subtract)
                    nc.vector.tensor_mul(planes[:, s, :], planes[:, s, :], s_bc)

                # permute partitions: transpose planes -> wT (free-interleave)
                # -> transpose back k-tile-wise into w_all. 8 transposes
                # share one fp16 PSUM bank; one strided copy drains it.
                for ot in range(OT):
                    wt = wt_pool.tile([P, PACK * P], dt.float16, tag="wt")
                    pstA = psumt_pool.tile([P, PACK * P], dt.float16,
                                           tag="pst")
                    for s in range(PACK):
                        nc.tensor.transpose(
                            pstA[:, s * P:(s + 1) * P],
                            planes[:, s, ot * P:(ot + 1) * P], ident)
                    # wt free dim = in-feature within row-tile = 8*wr + s
                    nc.vector.tensor_copy(
                        wt.rearrange("p (r s) -> p s r", s=PACK),
                        pstA.rearrange("p (s r) -> p s r", r=P))
                    # k-tiles covered by this row-tile: 8 per rt
                    pstB = psumt_pool.tile([P, PACK * P], dt.float16,
                                           tag="pst2")
                    for kk in range(PACK):
                        nc.tensor.transpose(
                            pstB[:, kk * P:(kk + 1) * P],
                            wt[:, kk * P:(kk + 1) * P], ident)
                    nc.vector.tensor_copy(
                        w_all[:, rt * PACK:(rt + 1) * PACK,
                              ot * P:(ot + 1) * P],
                        pstB.rearrange("p (kk r) -> p kk r", r=P))

            # ---- main loop: out[mseq, nout] = sum_k xT[k, m] * w[k, n] ----
            for cn in range(NCH):
                xts = []
                for k in range(KT):
                    xtk = xt_pool.tile([P, chunk], dt.float16, tag="xt")
                    nc.sync.dma_start(
                        xtk,
                        x[cn * chunk:(cn + 1) * chunk, k * P:(k + 1) * P],
                        transpose=True)
                    xts.append(xtk)
                for st in range(ST):
                    ps = psum_pool.tile([P, out_s], dt.float32, tag="mm")
                    for k in range(KT):
                        nc.tensor.matmul(
                            ps, lhsT=xts[k][:, st * P:(st + 1) * P],
                            rhs=w_all[:, k, :],
                            start=(k == 0), stop=(k == KT - 1))
                    o16 = out_pool.tile([P, out_s], dt.float16, tag="o16")
                    nc.vector.tensor_add(o16, ps, bias32)
                    r0 = cn * chunk + st * P
                    nc.gpsimd.dma_start(out[r0:r0 + P, :], o16)

    nc.compile()
    return nc


def _get_program(seq, in_f, out_s, chunk):
    key = (seq, in_f, out_s, chunk)
    if key not in _CACHE:
        _CACHE[key] = _build(seq, in_f, out_s, chunk)
    return _CACHE[key]


def kernel(x, qweight, scales, qzeros, g_idx=None, bias=None, **_unused):
    """Full-input entry point: shards over 8 cores, runs on HW, gathers."""
    from concourse.bass_utils import run_bass_kernel_spmd

    x = np.asarray(x)
    qweight = np.asarray(qweight)
    scales = np.asarray(scales)
    qzeros = np.asarray(qzeros)
    bias = np.asarray(bias)

    x2 = np.ascontiguousarray(x.reshape(SEQ, IN))
    nc = _get_program(SEQ, IN, OUT_S, 1024)

    zcols = OUT_S // PACK
    in_maps = []
    for c in range(NCORES):
        o0 = c * OUT_S
        in_maps.append({
            "x": x2,
            "qweight": np.ascontiguousarray(qweight[:, o0:o0 + OUT_S]),
            "scales": np.ascontiguousarray(scales[:, o0:o0 + OUT_S]),
            "qzeros": np.ascontiguousarray(qzeros[:, c * zcols:(c + 1) * zcols]),
            "bias": np.ascontiguousarray(bias[o0:o0 + OUT_S].reshape(1, OUT_S)),
        })

    res = run_bass_kernel_spmd(nc, in_maps, core_ids=list(range(NCORES)))
    full = np.concatenate([res.results[c]["out"] for c in range(NCORES)], axis=1)
    return full.reshape(B, S, OUT).astype(np.float16)


# revision 13
# speedup vs baseline: 1.1381x; 1.1381x over previous
"""GPTQ 4-bit quantized linear (CaiQuantLinear) on 8 TRN2 NeuronCores.

Computes out = x @ dequant(qweight, scales, qzeros) + bias where
  x: (4, 2048, 4096) fp16, qweight: (512, 4096) int32 (8x 4-bit per word,
  packed along input features), scales: (32, 4096) fp16, qzeros: (32, 512)
  int32 (packed along output features), bias: (4096,) fp16.
  Groups are contiguous blocks of 128 input features (g_idx = arange//128).

Sharding: tensor-parallel column split over output features. Each of the 8
cores gets 512 output columns (its slice of qweight/scales/qzeros/bias) and
the full x (replicated). No collectives; the host concatenates the 8 column
slices.

Per-core kernel:
  1. Unpack nibble planes of qweight in natural layout ([word-row, out]),
     dequantize against partition-broadcast (z+1) and scale tiles.
  2. Fix the partition permutation (plane s, word-row r -> input feature
     8r+s) with two PE transposes (planes -> w^T -> w), leaving fp16
     weights resident in SBUF as [128, 32 k-tiles, 512 out].
  3. Stream x through DMA-transpose into [128 in, seq-chunk] tiles and
     accumulate 32 matmuls per 128-seq output tile into PSUM; add bias on
     the way out.
"""

import sys

if "/opt/trn_rl_repo" not in sys.path:
    sys.path.insert(0, "/opt/trn_rl_repo")

import numpy as np

B, S, IN, OUT = 4, 2048, 4096, 4096
SEQ = B * S                      # 8192
NCORES = 8
OUT_S = OUT // NCORES            # 512 output columns per core
PACK = 8                         # int32 packs 8 nibbles
GSIZE = 128                      # group size == k-tile size

_CACHE = {}


def _build(seq, in_f, out_s, chunk):
    """Build + compile the per-core Bass program. All cores run the same
    NEFF on their own input slices (SPMD, no collectives)."""
    from contextlib import ExitStack  # noqa: F401

    import concourse.bass as bass  # noqa: F401
    import concourse.mybir as mybir
    import concourse.tile as tile
    from concourse import bacc
    from concourse.masks import make_identity

    dt = mybir.dt
    op = mybir.AluOpType
    P = 128
    KT = in_f // P                # k-tiles (== groups)
    QR = in_f // PACK             # qweight rows
    RT = QR // P                  # qweight row-tiles
    OT = out_s // P               # 128-wide output blocks per core
    NCH = seq // chunk            # seq chunks
    ST = chunk // P               # seq tiles per chunk

    nc = bacc.Bacc("TRN2", target_bir_lowering=False, debug=False,
                   num_devices=NCORES)

    x_d = nc.dram_tensor("x", (seq, in_f), dt.float16, kind="ExternalInput")
    qw_d = nc.dram_tensor("qweight", (QR, out_s), dt.int32, kind="ExternalInput")
    sc_d = nc.dram_tensor("scales", (KT, out_s), dt.float16, kind="ExternalInput")
    qz_d = nc.dram_tensor("qzeros", (KT, out_s // PACK), dt.int32,
                          kind="ExternalInput")
    b_d = nc.dram_tensor("bias", (1, out_s), dt.float16, kind="ExternalInput")
    out_d = nc.dram_tensor("out", (seq, out_s), dt.float16, kind="ExternalOutput")

    x = x_d.ap()
    qw = qw_d.ap()
    scales = sc_d.ap()
    qzeros = qz_d.ap()
    bias = b_d.ap()
    out = out_d.ap()

    with tile.TileContext(nc) as tc:
        with (
            tc.tile_pool(name="const", bufs=1) as const_pool,
            tc.tile_pool(name="w", bufs=1) as w_pool,
            tc.tile_pool(name="qst", bufs=3) as q_pool,
            tc.tile_pool(name="plane", bufs=2) as plane_pool,
            tc.tile_pool(name="wt", bufs=2) as wt_pool,
            tc.tile_pool(name="bc", bufs=2) as bc_pool,
            tc.tile_pool(name="xt", bufs=48) as xt_pool,
            tc.tile_pool(name="ot", bufs=4) as out_pool,
            tc.tile_pool(name="ps", bufs=4, space="PSUM") as psum_pool,
            tc.tile_pool(name="pst", bufs=2, space="PSUM") as psumt_pool,
            tc.tile_pool(name="dram", bufs=1, space="DRAM") as dram_pool,
        ):
            # ---- constants ----
            ident = const_pool.tile([P, P], dt.float16)
            make_identity(nc, ident)

            bias16 = const_pool.tile([P, out_s], dt.float16)
            nc.gpsimd.dma_start(bias16, bias.to_broadcast((P, out_s)))
            bias32 = const_pool.tile([P, out_s], dt.float32)
            nc.vector.tensor_copy(bias32, bias16)

            # ---- dequantize weights ----
            # w_all[:, k, :]: k-tile k of fp16 weights, [128 in x out_s]
            w_all = w_pool.tile([P, KT, out_s], dt.float16)
            for rt in range(RT):
                q_nat = q_pool.tile([P, out_s], dt.int32, tag="qnat")
                nc.gpsimd.dma_start(q_nat, qw[rt * P:(rt + 1) * P, :])

                # broadcast (z+1) and scale rows for this row-tile:
                # partition wr needs group 8*rt + wr//16. qzeros is loaded
                # directly in broadcast layout and unpacked on-chip.
                qzb = bc_pool.tile([P, out_s // PACK], dt.int32, tag="qzb")
                nc.gpsimd.dma_start(
                    qzb,
                    qzeros[8 * rt:8 * rt + 8][:, None, :]
                    .to_broadcast((8, 16, out_s // PACK)))
                zb_i = bc_pool.tile([P, out_s], dt.int32, tag="zbi")
                zbv = zb_i.rearrange("p (c s) -> p c s", s=PACK)
                for s in range(PACK):
                    nc.vector.tensor_scalar(
                        out=zbv[:, :, s], in0=qzb, scalar1=4 * s, scalar2=0xF,
                        op0=op.logical_shift_right, op1=op.bitwise_and)
                z1_bc = bc_pool.tile([P, out_s], dt.float16, tag="z1bc")
                nc.vector.tensor_scalar_add(z1_bc, zb_i, 1.0)
                s_bc = bc_pool.tile([P, out_s], dt.float16, tag="sbc")
                nc.gpsimd.dma_start(
                    s_bc,
                    scales[8 * rt:8 * rt + 8][:, None, :]
                    .to_broadcast((8, 16, out_s)))

                # unpack 8 nibble planes + dequant (still [word-row, out])
                planes = plane_pool.tile([P, PACK, out_s], dt.float16, tag="pl")
                for s in range(PACK):
                    plane_i = q_pool.tile([P, out_s], dt.int32, tag="plane_i")
                    nc.vector.tensor_scalar(
                        out=plane_i, in0=q_nat, scalar1=4 * s,
                        scalar2=0xF, op0=op.logical_shift_right,
                        op1=op.bitwise_and)
                    # fused cast + subtract: (plane + 0) - (z+1)
                    nc.vector.scalar_tensor_tensor(
                        out=planes[:, s, :], in0=plane_i, scalar=0.0,
                        in1=z1_bc, op0=op.add, op1=op.subtract)
                    nc.vector.tensor_mul(planes[:, s, :], planes[:, s, :], s_bc)

                # permute partitions: transpose planes -> wT (free-interleave)
                # -> transpose back k-tile-wise into w_all. 8 transposes
                # share one fp16 PSUM bank; one strided copy drains it.
                for ot in range(OT):
                    wt = wt_pool.tile([P, PACK * P], dt.float16, tag="wt")
                    pstA = psumt_pool.tile([P, PACK * P], dt.float16,
                                           tag="pst")
                    for s in range(PACK):
                        nc.tensor.transpose(
                            pstA[:, s * P:(s + 1) * P],
                            planes[:, s, ot * P:(ot + 1) * P], ident)
                    # wt free dim = in-feature within row-tile = 8*wr + s
                    nc.vector.tensor_copy(
                        wt.rearrange("p (r s) -> p s r", s=PACK),
                        pstA.rearrange("p (s r) -> p s r", r=P))
                    # k-tiles covered by this row-tile: 8 per rt
                    pstB = psumt_pool.tile([P, PACK * P], dt.float16,
                                           tag="pst2")
                    for kk in range(PACK):
                        nc.tensor.transpose(
                            pstB[:, kk * P:(kk + 1) * P],
                            wt[:, kk * P:(kk + 1) * P], ident)
                    nc.vector.tensor_copy(
                        w_all[:, rt * PACK:(rt + 1) * PACK,
                              ot * P:(ot + 1) * P],
                        pstB.rearrange("p (kk r) -> p kk r", r=P))

            # ---- main loop: out[mseq, nout] = sum_k xT[k, m] * w[k, n] ----
            for cn in range(NCH):
                xts = []
                for k in range(KT):
                    xtk = xt_pool.tile([P, chunk], dt.float16, tag="xt")
                    nc.sync.dma_start(
                        xtk,
                        x[cn * chunk:(cn + 1) * chunk, k * P:(k + 1) * P],
                        transpose=True)
                    xts.append(xtk)
                for st in range(ST):
                    ps = psum_pool.tile([P, out_s], dt.float32, tag="mm")
                    for k in range(KT):
                        nc.tensor.matmul(
                            ps, lhsT=xts[k][:, st * P:(st + 1) * P],
                            rhs=w_all[:, k, :],
                            start=(k == 0), stop=(k == KT - 1))
                    o16 = out_pool.tile([P, out_s], dt.float16, tag="o16")
                    nc.vector.tensor_add(o16, ps, bias32)
                    r0 = cn * chunk + st * P
                    nc.gpsimd.dma_start(out[r0:r0 + P, :], o16)

    nc.compile()
    return nc


def _get_program(seq, in_f, out_s, chunk):
    key = (seq, in_f, out_s, chunk)
    if key not in _CACHE:
        _CACHE[key] = _build(seq, in_f, out_s, chunk)
    return _CACHE[key]


def kernel(x, qweight, scales, qzeros, g_idx=None, bias=None, **_unused):
    """Full-input entry point: shards over 8 cores, runs on HW, gathers."""
    from concourse.bass_utils import run_bass_kernel_spmd

    x = np.asarray(x)
    qweight = np.asarray(qweight)
    scales = np.asarray(scales)
    qzeros = np.asarray(qzeros)
    bias = np.asarray(bias)

    x2 = np.ascontiguousarray(x.reshape(SEQ, IN))
    nc = _get_program(SEQ, IN, OUT_S, 1024)

    zcols = OUT_S // PACK
    in_maps = []
    for c in range(NCORES):
        o0 = c * OUT_S
        in_maps.append({
            "x": x2,
            "qweight": np.ascontiguousarray(qweight[:, o0:o0 + OUT_S]),
            "scales": np.ascontiguousarray(scales[:, o0:o0 + OUT_S]),
            "qzeros": np.ascontiguousarray(qzeros[:, c * zcols:(c + 1) * zcols]),
            "bias": np.ascontiguousarray(bias[o0:o0 + OUT_S].reshape(1, OUT_S)),
        })

    res = run_bass_kernel_spmd(nc, in_maps, core_ids=list(range(NCORES)))
    full = np.concatenate([res.results[c]["out"] for c in range(NCORES)], axis=1)
    return full.reshape(B, S, OUT).astype(np.float16)
